# revision 1
# baseline (speedup 1.0000x reference)
"""3-layer GAT (graph attention network) forward pass on 8 Trainium2 cores.

Strategy (graph/data parallel per the destination-node partition):
  - Nodes are partitioned contiguously across 8 cores (12500 each), then
    re-binned within each core into 98 blocks of 128 nodes with balanced
    in-degree (LPT), so every (core, block) has the same padded edge-tile
    count T. The SPMD program is therefore identical on all cores; only the
    data (index arrays) differs.
  - Per layer: each core computes its own chunk of the gather table
    [W@h | el] ([feat2|el2] for the output layer) and AllGathers it, plus a
    core-local er table. Edge phase: per 128-node block, indirect-DMA gather
    of per-edge source rows by src index, per-edge er by dst index, then
    ee = exp(leaky_relu(el_src + er_dst)) and a one-hot segment matmul that
    accumulates [sum(ee*feat) | sum(ee)] over the block's edges in PSUM.
    Softmax normalization (divide by sum(ee)) happens per node afterwards —
    numerically safe here because |e| stays small, so no segment-max shift.
  - Layers 0/1 keep everything transposed ([channel, node]) so BN reduces
    along the free axis and BN+ReLU applies as one activation op with
    per-partition scale/bias; BN makes the +b0/+b1 biases no-ops, so they
    are dropped. BN stats are AllReduced across cores.
  - Layer 2 runs node-major and writes output rows directly.
  - One NEFF per layer: walrus tracks SWDGE DMA-queue completion counts in a
    16-bit ISA field across a whole NEFF (~4096 indirect DMAs max), so the
    three layers are separate programs; h is handed between them by the host.
"""

import heapq
import numpy as np

import concourse.bass as bass
import concourse.bacc as bacc
import concourse.tile as tile
from concourse import mybir, bass_utils

F32 = mybir.dt.float32
I32 = mybir.dt.int32
BF16 = mybir.dt.bfloat16

# Layers 0/1 gather-table rows stored as [Wh bf16 | el f32-bitcast]:
# halves the dominant indirect-gather bytes; attention logits stay f32.
TBL_BF16 = False


class Cfg:
    def __init__(self, n, e, ncores, in_dim=128, hid=16, heads=4, outc=40,
                 neg=0.2, eps=1e-5):
        assert n % ncores == 0
        self.N, self.E, self.NCORES = n, e, ncores
        self.IN, self.HID, self.HEADS, self.OUTC = in_dim, hid, heads, outc
        self.F = heads * hid          # 64
        self.F2 = heads * outc        # 160
        self.P = 128
        self.NPC = n // ncores        # real nodes per core
        self.BLOCKS = (self.NPC + self.P - 1) // self.P
        self.NOWN = self.BLOCKS * self.P   # padded nodes per core
        self.NPAD = ncores * self.NOWN
        self.NEG, self.EPS = neg, eps


CFG = Cfg(100000, 1600000, 8)


# ---------------------------------------------------------------- host prep

def _bin_nodes(cfg, deg):
    """LPT-bin each core's nodes into BLOCKS bins of P slots, balancing
    in-degree sums. Returns newlocal[node] = padded local id on its core."""
    newlocal = np.empty(cfg.N, np.int64)
    # capacity: last bin holds only the remainder so the unfilled (pad)
    # slots are exactly local ids [NPC, NOWN) — the device zeroes that range
    cap = np.full(cfg.BLOCKS, cfg.P, np.int64)
    cap[-1] = cfg.NPC - (cfg.BLOCKS - 1) * cfg.P
    for r in range(cfg.NCORES):
        lo, hi = r * cfg.NPC, (r + 1) * cfg.NPC
        d = deg[lo:hi]
        order = np.argsort(-d, kind="stable")
        heap = [(0, b) for b in range(cfg.BLOCKS)]
        heapq.heapify(heap)
        slots = np.zeros(cfg.BLOCKS, np.int64)
        loc = np.empty(cfg.NPC, np.int64)
        for i in order:
            while True:
                c, b = heapq.heappop(heap)
                if slots[b] < cap[b]:
                    break
            loc[i] = b * cfg.P + slots[b]
            slots[b] += 1
            heapq.heappush(heap, (c + int(d[i]), b))
        newlocal[lo:hi] = loc
    return newlocal


def preprocess(cfg, src, dst):
    """Static graph preprocessing. Returns per-core index arrays + T + perm."""
    src = np.asarray(src, np.int64)
    dst = np.asarray(dst, np.int64)
    deg = np.bincount(dst, minlength=cfg.N)
    newlocal = _bin_nodes(cfg, deg)
    core_of = np.arange(cfg.N) // cfg.NPC
    newglobal = core_of * cfg.NOWN + newlocal

    r_e = dst // cfg.NPC
    dloc = newlocal[dst]
    b_e = dloc // cfg.P
    slot_e = dloc % cfg.P
    s_glob = newglobal[src]

    key = r_e * cfg.BLOCKS + b_e
    order_e = np.argsort(key, kind="stable")
    cnt = np.bincount(key, minlength=cfg.NCORES * cfg.BLOCKS)
    T = int(np.ceil(cnt.max() / cfg.P))
    starts = np.concatenate([[0], np.cumsum(cnt)])

    se = s_glob[order_e]
    sl = slot_e[order_e]
    dl = dloc[order_e]

    TP = T * cfg.P
    srcidx = np.zeros((cfg.NCORES, cfg.BLOCKS, TP), np.int32)
    eridx = np.full((cfg.NCORES, cfg.BLOCKS, TP), cfg.NOWN, np.int32)
    colv = np.full((cfg.NCORES, cfg.BLOCKS, TP), -1.0, np.float32)
    for r in range(cfg.NCORES):
        for b in range(cfg.BLOCKS):
            k = r * cfg.BLOCKS + b
            c0, c1 = starts[k], starts[k + 1]
            n = c1 - c0
            srcidx[r, b, :n] = se[c0:c1]
            eridx[r, b, :n] = dl[c0:c1]
            colv[r, b, :n] = sl[c0:c1]

    # [r, b, T*P] -> [r, b, P, T]: edge k of a block -> tile k//P, partition k%P
    srcidx = np.ascontiguousarray(srcidx.reshape(cfg.NCORES, cfg.BLOCKS, T, cfg.P).transpose(0, 1, 3, 2))
    eridx = np.ascontiguousarray(eridx.reshape(cfg.NCORES, cfg.BLOCKS, T, cfg.P).transpose(0, 1, 3, 2))
    colv = np.ascontiguousarray(colv.reshape(cfg.NCORES, cfg.BLOCKS, T, cfg.P).transpose(0, 1, 3, 2))
    return srcidx, eridx, colv, T, newlocal


def _fold_el(W, a, heads, dph):
    # w[k, h] = sum_d W[k, h*dph+d] * a[h, d]
    return np.einsum("khd,hd->kh", W.reshape(W.shape[0], heads, dph), a).astype(np.float32)


# ---------------------------------------------------------------- program

def build_layer_program(cfg, T, layer):
    P, F, F2, IN = cfg.P, cfg.F, cfg.F2, cfg.IN
    H, D, OC = cfg.HEADS, cfg.HID, cfg.OUTC
    B = cfg.BLOCKS
    fo = F2 if layer == 2 else F
    fin = IN if layer == 0 else F
    nc = bacc.Bacc("TRN2", target_bir_lowering=False, debug=False,
                   num_devices=cfg.NCORES)

    def inp(name, shape, dt=F32):
        return nc.dram_tensor(name, shape, dt, kind="ExternalInput").ap()

    x_in = inp("x_in", [fin, cfg.NOWN])
    srcidx = inp("srcidx", [B, P, T], I32)
    eridx = inp("eridx", [B, P, T], I32)
    colv = inp("colv", [B, P, T])
    Wext_i = inp("Wext", [fin, fo + 8])
    iota_in = inp("iota", [P, P])
    if layer < 2:
        gamma_i = inp("gamma", [F, 1])
        beta_i = inp("beta", [F, 1])
        bsel_in = inp("bsel", [H, F])
        y_out = nc.dram_tensor("y_out", [F, cfg.NOWN], F32,
                               kind="ExternalOutput").ap()
    if layer == 0:
        resW_i = inp("resW", [IN, F])
    if layer == 2:
        resW_i = inp("resW", [F, F2])
        b2bc_in = inp("b2bc", [P, F2])
        out = nc.dram_tensor("out", [cfg.NOWN, F2], F32,
                             kind="ExternalOutput").ap()

    rg = [list(range(cfg.NCORES))]

    with tile.TileContext(nc) as tc:
        with (
            tc.tile_pool(name="big", bufs=1) as bigp,
            tc.tile_pool(name="const", bufs=1) as cons,
            tc.tile_pool(name="work", bufs=3) as wk,
            tc.tile_pool(name="ps", bufs=2, space="PSUM") as ps,
            tc.tile_pool(name="dram", bufs=1, space="DRAM") as dr,
        ):
            # DRAM scratch
            bf = TBL_BF16 and layer < 2
            tdt = BF16 if bf else F32
            fw = fo + 8 if bf else fo + 4  # gather row width (tdt units)
            tbl_own = dr.tile([cfg.NOWN, fw], tdt)
            tbl_full = dr.tile([cfg.NPAD, fw], tdt, addr_space="Shared")
            er_dram = dr.tile([cfg.NOWN + 1, 4], F32)
            if layer < 2:
                stats_in = dr.tile([F, 2], F32)
                stats_out = dr.tile([F, 2], F32, addr_space="Shared")

            # per-layer SBUF state
            xT = bigp.tile([fin, cfg.NOWN], F32, tag="xT")
            nc.sync.dma_start(out=xT[:], in_=x_in[:])
            if layer < 2:
                yT = bigp.tile([F, cfg.NOWN], F32, tag="yT")

            Wext = cons.tile([fin, fo + 8], F32, tag="Wext")
            nc.sync.dma_start(out=Wext[:], in_=Wext_i[:])
            iota_f = cons.tile([P, P], F32, tag="iota_sb")
            nc.sync.dma_start(out=iota_f[:], in_=iota_in[:])
            if layer < 2:
                bsel = cons.tile([H, F], F32, tag="bsel_sb")
                nc.sync.dma_start(out=bsel[:], in_=bsel_in[:])
                gam = cons.tile([F, 1], F32, tag="gam")
                bet = cons.tile([F, 1], F32, tag="bet")
                nc.sync.dma_start(out=gam[:], in_=gamma_i[:])
                nc.sync.dma_start(out=bet[:], in_=beta_i[:])
                stats = cons.tile([F, 2], F32, tag="stats")
                nc.vector.memset(stats[:], 0.0)
            if layer == 0:
                rW = cons.tile([IN, F], F32, tag="rW")
                nc.sync.dma_start(out=rW[:], in_=resW_i[:])
            if layer == 2:
                rW = cons.tile([F, F2], F32, tag="rW")
                nc.sync.dma_start(out=rW[:], in_=resW_i[:])
                b2bc = cons.tile([P, F2], F32, tag="b2bc_sb")
                nc.sync.dma_start(out=b2bc[:], in_=b2bc_in[:])
            padrow = cons.tile([1, 4], F32, tag="padrow")
            nc.vector.memset(padrow[:], -1e38)

            # ---- phase A: gather tables + AllGather ----
            for b in range(B):
                tp = ps.tile([P, fo + 8], F32, tag="tbl")
                nc.tensor.matmul(out=tp[:], lhsT=xT[:, b * P:(b + 1) * P],
                                 rhs=Wext[:], start=True, stop=True)
                if bf:
                    rowb = wk.tile([P, fo + 8], BF16, tag="trow")
                    nc.vector.tensor_copy(out=rowb[:, :fo], in_=tp[:, :fo])
                    nc.vector.tensor_copy(
                        out=rowb[:, fo:fo + 8].bitcast(F32),
                        in_=tp[:, fo:fo + 4])
                    erow = wk.tile([P, 4], F32, tag="erow")
                    nc.vector.tensor_copy(out=erow[:], in_=tp[:, fo + 4:fo + 8])
                    nc.sync.dma_start(out=tbl_own[b * P:(b + 1) * P, :],
                                      in_=rowb[:])
                    nc.sync.dma_start(out=er_dram[b * P:(b + 1) * P, :],
                                      in_=erow[:])
                else:
                    row = wk.tile([P, fo + 8], F32, tag="trow")
                    nc.vector.tensor_copy(out=row[:], in_=tp[:])
                    nc.sync.dma_start(out=tbl_own[b * P:(b + 1) * P, :],
                                      in_=row[:, :fo + 4])
                    nc.sync.dma_start(out=er_dram[b * P:(b + 1) * P, :],
                                      in_=row[:, fo + 4:fo + 8])
            nc.sync.dma_start(out=er_dram[cfg.NOWN:cfg.NOWN + 1, :],
                              in_=padrow[:])
            nc.gpsimd.collective_compute(
                "AllGather", mybir.AluOpType.bypass, replica_groups=rg,
                ins=[tbl_own[:].opt()], outs=[tbl_full[:].opt()])
            # gathers race the collective's remote writes without this:
            tc.strict_bb_all_engine_barrier()

            # ---- phase B: edge aggregation per block ----
            fo4 = fo + 4
            dph = OC if layer == 2 else D
            for b in range(B):
                sidx = wk.tile([P, T], I32, tag="sidx")
                eidx = wk.tile([P, T], I32, tag="eidx")
                colf = wk.tile([P, T], F32, tag="colf")
                nc.sync.dma_start(out=sidx[:], in_=srcidx[b])
                nc.sync.dma_start(out=eidx[:], in_=eridx[b])
                nc.sync.dma_start(out=colf[:], in_=colv[b])

                G = wk.tile([P, T * fw], tdt, tag="G")
                R = wk.tile([P, T * 4], F32, tag="R")
                for t in range(T):
                    nc.gpsimd.indirect_dma_start(
                        out=G[:, t * fw:(t + 1) * fw], out_offset=None,
                        in_=tbl_full[:],
                        in_offset=bass.IndirectOffsetOnAxis(
                            ap=sidx[:, t:t + 1], axis=0))
                    nc.gpsimd.indirect_dma_start(
                        out=R[:, t * 4:(t + 1) * 4], out_offset=None,
                        in_=er_dram[:],
                        in_offset=bass.IndirectOffsetOnAxis(
                            ap=eidx[:, t:t + 1], axis=0))

                G3 = G[:].rearrange("p (t f) -> p t f", t=T)
                el_view = (G3[:, :, fo:fo + 8].bitcast(F32) if bf
                           else G3[:, :, fo:fo4])
                ee = wk.tile([P, T * H], F32, tag="ee")
                ee3 = ee[:].rearrange("p (t h) -> p t h", t=T)
                nc.vector.tensor_tensor(
                    out=ee3, in0=el_view,
                    in1=R[:].rearrange("p (t h) -> p t h", t=T),
                    op=mybir.AluOpType.add)
                esc = wk.tile([P, T * H], F32, tag="esc")
                nc.vector.tensor_scalar_mul(out=esc[:], in0=ee[:],
                                            scalar1=cfg.NEG)
                nc.vector.tensor_tensor(out=ee[:], in0=ee[:], in1=esc[:],
                                        op=mybir.AluOpType.max)
                nc.scalar.activation(out=ee[:], in_=ee[:],
                                     func=mybir.ActivationFunctionType.Exp)

                O = wk.tile([P, T * P], F32, tag="O")
                nc.vector.tensor_tensor(
                    out=O[:].rearrange("p (t c) -> p t c", t=T),
                    in0=iota_f[:].unsqueeze(1).to_broadcast([P, T, P]),
                    in1=colf[:].unsqueeze(2).to_broadcast([P, T, P]),
                    op=mybir.AluOpType.is_equal)

                V = wk.tile([P, T * fo4], F32, tag="V")
                V3 = V[:].rearrange("p (t f) -> p t f", t=T)
                nc.vector.tensor_tensor(
                    out=V3[:, :, :fo].rearrange("p t (h d) -> p t h d", h=H),
                    in0=G3[:, :, :fo].rearrange("p t (h d) -> p t h d", h=H),
                    in1=ee3.unsqueeze(3).to_broadcast([P, T, H, dph]),
                    op=mybir.AluOpType.mult)
                nc.vector.tensor_copy(out=V3[:, :, fo:fo4], in_=ee3)

                if layer < 2:
                    # transposed accumulate: acc[fo+4, dst] += V_t.T @ O_t
                    acc = ps.tile([F + 4, P], F32, tag="acc")
                    for t in range(T):
                        nc.tensor.matmul(
                            out=acc[:], lhsT=V[:, t * fo4:(t + 1) * fo4],
                            rhs=O[:, t * P:(t + 1) * P],
                            start=(t == 0), stop=(t == T - 1))
                    dmax = wk.tile([H, P], F32, tag="dmax")
                    nc.vector.tensor_scalar_max(out=dmax[:],
                                                in0=acc[F:F + 4, :],
                                                scalar1=1e-16)
                    rec = wk.tile([H, P], F32, tag="rec")
                    nc.vector.reciprocal(out=rec[:], in_=dmax[:])
                    recb_ps = ps.tile([F, P], F32, tag="aux")
                    nc.tensor.matmul(out=recb_ps[:], lhsT=bsel[:],
                                     rhs=rec[:], start=True, stop=True)
                    recb = wk.tile([F, P], F32, tag="recb")
                    nc.vector.tensor_copy(out=recb[:], in_=recb_ps[:])

                    hsl = yT[:, b * P:(b + 1) * P]
                    nc.vector.tensor_tensor(out=hsl, in0=acc[:F, :],
                                            in1=recb[:],
                                            op=mybir.AluOpType.mult)
                    if layer == 0:
                        res_ps = ps.tile([F, P], F32, tag="aux")
                        nc.tensor.matmul(out=res_ps[:], lhsT=rW[:],
                                         rhs=xT[:, b * P:(b + 1) * P],
                                         start=True, stop=True)
                        nc.vector.tensor_tensor(out=hsl, in0=hsl,
                                                in1=res_ps[:],
                                                op=mybir.AluOpType.add)
                    else:
                        hold = xT[:, b * P:(b + 1) * P]
                        nc.vector.tensor_tensor(out=hsl, in0=hsl, in1=hold,
                                                op=mybir.AluOpType.add)
                        nc.vector.tensor_tensor(out=hsl, in0=hsl, in1=hold,
                                                op=mybir.AluOpType.add)

                    red = wk.tile([F, 1], F32, tag="red")
                    nc.vector.reduce_sum(out=red[:], in_=hsl,
                                         axis=mybir.AxisListType.X)
                    nc.vector.tensor_tensor(out=stats[:, 0:1],
                                            in0=stats[:, 0:1], in1=red[:],
                                            op=mybir.AluOpType.add)
                    sq = wk.tile([F, P], F32, tag="sq")
                    nc.scalar.activation(
                        out=sq[:], in_=hsl,
                        func=mybir.ActivationFunctionType.Square)
                    nc.vector.reduce_sum(out=red[:], in_=sq[:],
                                         axis=mybir.AxisListType.X)
                    nc.vector.tensor_tensor(out=stats[:, 1:2],
                                            in0=stats[:, 1:2], in1=red[:],
                                            op=mybir.AluOpType.add)
                else:
                    # node-major accumulate: acc[dst, fo+4] += O_t.T @ V_t
                    acc = ps.tile([P, F2 + 4], F32, tag="acc")
                    for t in range(T):
                        nc.tensor.matmul(
                            out=acc[:], lhsT=O[:, t * P:(t + 1) * P],
                            rhs=V[:, t * fo4:(t + 1) * fo4],
                            start=(t == 0), stop=(t == T - 1))
                    dmax = wk.tile([P, H], F32, tag="dmax")
                    nc.vector.tensor_scalar_max(out=dmax[:],
                                                in0=acc[:, F2:F2 + 4],
                                                scalar1=1e-16)
                    rec = wk.tile([P, H], F32, tag="rec")
                    nc.vector.reciprocal(out=rec[:], in_=dmax[:])
                    rst = wk.tile([P, F2], F32, tag="rst")
                    nc.vector.tensor_tensor(
                        out=rst[:].rearrange("p (h o) -> p h o", h=H),
                        in0=acc[:, :F2].rearrange("p (h o) -> p h o", h=H),
                        in1=rec[:].unsqueeze(2).to_broadcast([P, H, OC]),
                        op=mybir.AluOpType.mult)
                    res_ps = ps.tile([P, F2], F32, tag="aux")
                    nc.tensor.matmul(out=res_ps[:],
                                     lhsT=xT[:, b * P:(b + 1) * P],
                                     rhs=rW[:], start=True, stop=True)
                    nc.vector.tensor_tensor(out=rst[:], in0=rst[:],
                                            in1=res_ps[:],
                                            op=mybir.AluOpType.add)
                    nc.vector.tensor_tensor(out=rst[:], in0=rst[:],
                                            in1=b2bc[:],
                                            op=mybir.AluOpType.add)
                    nc.sync.dma_start(out=out[b * P:(b + 1) * P, :],
                                      in_=rst[:])

            # ---- phase C: BN (layers 0/1) + writeback ----
            if layer < 2:
                nc.sync.dma_start(out=stats_in[:], in_=stats[:])
                nc.gpsimd.collective_compute(
                    "AllReduce", mybir.AluOpType.add, replica_groups=rg,
                    ins=[stats_in[:].opt()], outs=[stats_out[:].opt()])
                tc.strict_bb_all_engine_barrier()
                ssb = cons.tile([F, 2], F32, tag="ssb")
                nc.sync.dma_start(out=ssb[:], in_=stats_out[:])
                mu = cons.tile([F, 1], F32, tag="mu")
                tmp1 = cons.tile([F, 1], F32, tag="tmp1")
                scl = cons.tile([F, 1], F32, tag="scl")
                bia = cons.tile([F, 1], F32, tag="bia")
                musq = cons.tile([F, 1], F32, tag="musq")
                invn = 1.0 / cfg.N
                nc.vector.tensor_scalar_mul(out=mu[:], in0=ssb[:, 0:1],
                                            scalar1=invn)
                nc.vector.tensor_scalar_mul(out=tmp1[:], in0=ssb[:, 1:2],
                                            scalar1=invn)
                nc.scalar.activation(out=musq[:], in_=mu[:],
                                     func=mybir.ActivationFunctionType.Square)
                nc.vector.tensor_tensor(out=tmp1[:], in0=tmp1[:],
                                        in1=musq[:],
                                        op=mybir.AluOpType.subtract)
                nc.vector.tensor_scalar_add(out=tmp1[:], in0=tmp1[:],
                                            scalar1=cfg.EPS)
                nc.scalar.activation(out=tmp1[:], in_=tmp1[:],
                                     func=mybir.ActivationFunctionType.Sqrt)
                nc.vector.reciprocal(out=tmp1[:], in_=tmp1[:])
                nc.vector.tensor_tensor(out=scl[:], in0=tmp1[:], in1=gam[:],
                                        op=mybir.AluOpType.mult)
                nc.vector.tensor_tensor(out=tmp1[:], in0=mu[:], in1=scl[:],
                                        op=mybir.AluOpType.mult)
                nc.vector.tensor_tensor(out=bia[:], in0=bet[:], in1=tmp1[:],
                                        op=mybir.AluOpType.subtract)
                nc.scalar.activation(out=yT[:, :], in_=yT[:, :],
                                     func=mybir.ActivationFunctionType.Relu,
                                     scale=scl[:, 0:1], bias=bia[:, 0:1])
                if cfg.NPC < cfg.NOWN:
                    nc.vector.memset(yT[:, cfg.NPC:cfg.NOWN], 0.0)
                nc.sync.dma_start(out=y_out[:], in_=yT[:])

    nc.compile()
    return nc


# ---------------------------------------------------------------- host glue

def make_in_maps(cfg, inputs, srcidx, eridx, colv, newlocal):
    feat = np.asarray(inputs["feat"], np.float32)
    H, D, OC = cfg.HEADS, cfg.HID, cfg.OUTC
    W0 = np.asarray(inputs["W0"], np.float32)
    W1 = np.asarray(inputs["W1"], np.float32)
    W2 = np.asarray(inputs["W2"], np.float32)
    W0ext = np.concatenate([W0, _fold_el(W0, np.asarray(inputs["al0"]), H, D),
                            _fold_el(W0, np.asarray(inputs["ar0"]), H, D)], axis=1)
    W1ext = np.concatenate([W1, _fold_el(W1, np.asarray(inputs["al1"]), H, D),
                            _fold_el(W1, np.asarray(inputs["ar1"]), H, D)], axis=1)
    W2ext = np.concatenate([W2, _fold_el(W2, np.asarray(inputs["al2"]), H, OC),
                            _fold_el(W2, np.asarray(inputs["ar2"]), H, OC)], axis=1)

    common = {
        "bsel": np.repeat(np.eye(cfg.HEADS, dtype=np.float32), cfg.HID, axis=1),
        "iota": np.tile(np.arange(cfg.P, dtype=np.float32)[None, :], (cfg.P, 1)),
    }
    layer_maps = [[], [], []]
    for r in range(cfg.NCORES):
        ids = np.arange(r * cfg.NPC, (r + 1) * cfg.NPC)
        fp = np.zeros((cfg.NOWN, cfg.IN), np.float32)
        fp[newlocal[ids]] = feat[ids]
        idx = {"srcidx": srcidx[r], "eridx": eridx[r], "colv": colv[r]}
        layer_maps[0].append({
            "x_in": np.ascontiguousarray(fp.T), "Wext": W0ext,
            "resW": np.asarray(inputs["resW0"], np.float32),
            "gamma": np.asarray(inputs["gamma0"], np.float32).reshape(cfg.F, 1),
            "beta": np.asarray(inputs["beta0"], np.float32).reshape(cfg.F, 1),
            **idx, **common,
        })
        layer_maps[1].append({
            "Wext": W1ext,
            "gamma": np.asarray(inputs["gamma1"], np.float32).reshape(cfg.F, 1),
            "beta": np.asarray(inputs["beta1"], np.float32).reshape(cfg.F, 1),
            **idx, **common,
        })
        layer_maps[2].append({
            "Wext": W2ext,
            "resW": np.asarray(inputs["resW2"], np.float32),
            "b2bc": np.tile(np.asarray(inputs["b2"], np.float32).reshape(1, cfg.F2),
                            (cfg.P, 1)),
            "iota": common["iota"],
            **idx,
        })
    return layer_maps


def assemble_output(cfg, results, newlocal):
    out = np.empty((cfg.N, cfg.F2), np.float32)
    for r in range(cfg.NCORES):
        ids = np.arange(r * cfg.NPC, (r + 1) * cfg.NPC)
        out[ids] = results[r]["out"][newlocal[ids]]
    return out


_PROG_CACHE = {}


def get_program(cfg, T, layer):
    key = (cfg.N, cfg.E, T, layer, TBL_BF16)
    if key not in _PROG_CACHE:
        _PROG_CACHE[key] = build_layer_program(cfg, T, layer)
    return _PROG_CACHE[key]


def run(inputs, trace=False, trace_cores=None):
    cfg = CFG
    src = np.asarray(inputs["src"])
    dst = np.asarray(inputs["dst"])
    srcidx, eridx, colv, T, newlocal = preprocess(cfg, src, dst)
    layer_maps = make_in_maps(cfg, inputs, srcidx, eridx, colv, newlocal)
    cores = list(range(cfg.NCORES))
    total_ns = 0
    layer_res = []
    for layer in range(3):
        nc = get_program(cfg, T, layer)
        res = bass_utils.run_bass_kernel_spmd(
            nc, layer_maps[layer], core_ids=cores,
            trace=trace, trace_cores=trace_cores)
        layer_res.append(res)
        if res.exec_time_ns:
            total_ns += res.exec_time_ns
        if layer < 2:
            for r in range(cfg.NCORES):
                layer_maps[layer + 1][r]["x_in"] = res.results[r]["y_out"]
    out = assemble_output(cfg, layer_res[2].results, newlocal)
    return out, (total_ns, layer_res)


def kernel(**inputs) -> np.ndarray:
    return run(inputs)[0]



# revision 24
# speedup vs baseline: 1.1887x; 1.1887x over previous
"""3-layer GAT forward pass on 8 Trainium2 cores (v2: batched dma_gather).

Strategy (graph/data parallel by destination-node partition):
  - Nodes partitioned contiguously across 8 cores (12500 each), re-binned
    per core into 98 blocks of 128 nodes with balanced in-degree (LPT), so
    every (core, block) has a uniform padded edge-tile layout. The SPMD
    program is identical on all cores; only index data differs.
  - Per layer, each core computes a gather table [Wh bf16 | el f32-bitcast]
    for its own nodes (256B rows for layers 0/1, 512B for layer 2) and
    AllGathers it. Per-edge source rows are then fetched with batched
    dma_gather (InstDMAGatherAnt): one instruction gathers thousands of
    rows, vs one 128-row indirect DMA per tile in v1 — SWDGE fixed cost
    (~1us/instruction) dominated v1. int16 gather indices limit a call to
    32K rows, so edges are bucketed into 4 windows of 25088 table rows
    (2 source cores each); per (group-of-blocks, window) one call.
  - Per-edge er values are gathered the same way from a local per-node
    table (wide 256B rows; one call per group).
  - ee = exp(leaky_relu(el+er)) (fused DVE + ACT); V = [ee*feat | ee]
    (bf16); per-block one-hot segment matmul accumulates
    [sum(ee*feat) | sum(ee)] over tiles in PSUM. One-hot matrices are
    static per graph: precomputed on host (bf16) and DMA-loaded.
  - Softmax normalization is deferred: raw sums and denominators are
    staged to SBUF, then one big reciprocal + per-block PE broadcast
    matmul normalizes all nodes (avoids tiny per-block DVE ops).
  - Layers 0/1 run channel-major so BN reduces along the free axis; BN
    stats AllReduced; biases b0/b1 are no-ops under BN and dropped.
    Layer 2 runs node-major and writes output rows directly.
"""

import os
import numpy as np
import ml_dtypes

_STAGE = int(os.environ.get("GAT_STAGE", "9"))   # debug bisect level
_MAXG = int(os.environ.get("GAT_MAXG", "9999"))  # groups that issue gathers
_NOER = bool(int(os.environ.get("GAT_NOER", "0")))
_NOSRC = bool(int(os.environ.get("GAT_NOSRC", "0")))

import concourse.bass as bass
import concourse.bacc as bacc
import concourse.tile as tile
from concourse import mybir, bass_utils
from concourse.library_config import mlp

F32 = mybir.dt.float32
I32 = mybir.dt.int32
I16 = mybir.dt.int16
BF16 = mybir.dt.bfloat16
BF = ml_dtypes.bfloat16


class Cfg:
    def __init__(self, n, e, ncores, in_dim=128, hid=16, heads=4, outc=40,
                 neg=0.2, eps=1e-5):
        assert n % ncores == 0
        self.N, self.E, self.NCORES = n, e, ncores
        self.IN, self.HID, self.HEADS, self.OUTC = in_dim, hid, heads, outc
        self.F = heads * hid          # 64
        self.F2 = heads * outc        # 160
        self.P = 128
        self.NPC = n // ncores        # real nodes per core
        self.BLOCKS = (self.NPC + self.P - 1) // self.P
        self.NOWN = self.BLOCKS * self.P   # padded nodes per core
        self.NPAD = ncores * self.NOWN
        self.W = 2 * self.NOWN        # gather window rows (2 cores)
        self.NCH = self.NPAD // self.W     # 4 windows
        self.NEG, self.EPS = neg, eps


CFG = Cfg(100000, 1600000, 8)
NB1 = 3   # blocks per gather group, layers 0/1
NB2 = 3   # blocks per gather group, layer 2


# ---------------------------------------------------------------- host prep

def _bin_nodes(cfg, deg):
    """LPT-bin each core's nodes into BLOCKS bins of P slots, balancing
    in-degree sums. Returns newlocal[node] = padded local id on its core."""
    newlocal = np.empty(cfg.N, np.int64)
    cap = np.full(cfg.BLOCKS, cfg.P, np.int64)
    cap[-1] = cfg.NPC - (cfg.BLOCKS - 1) * cfg.P
    import heapq
    for r in range(cfg.NCORES):
        lo, hi = r * cfg.NPC, (r + 1) * cfg.NPC
        d = deg[lo:hi]
        order = np.argsort(-d, kind="stable")
        heap = [(0, b) for b in range(cfg.BLOCKS)]
        heapq.heapify(heap)
        slots = np.zeros(cfg.BLOCKS, np.int64)
        loc = np.empty(cfg.NPC, np.int64)
        for i in order:
            while True:
                c, b = heapq.heappop(heap)
                if slots[b] < cap[b]:
                    break
            loc[i] = b * cfg.P + slots[b]
            slots[b] += 1
            heapq.heappush(heap, (c + int(d[i]), b))
        newlocal[lo:hi] = loc
    return newlocal


def _groups(nblocks, nb):
    gs, b = [], 0
    while b < nblocks:
        gs.append(list(range(b, min(b + nb, nblocks))))
        b += nb
    return gs


def _wrap16(a):
    """[n] int16 -> dma_gather wrapped layout [128, n//16]."""
    w = a.reshape(-1, 16).T.astype(np.int16)
    return np.ascontiguousarray(np.tile(w, (8, 1)))


class Prep:
    pass


def preprocess(cfg, src, dst):
    src = np.asarray(src, np.int64)
    dst = np.asarray(dst, np.int64)
    deg = np.bincount(dst, minlength=cfg.N)
    newlocal = _bin_nodes(cfg, deg)
    core_of = np.arange(cfg.N) // cfg.NPC
    newglobal = core_of * cfg.NOWN + newlocal

    P, B, NCH = cfg.P, cfg.BLOCKS, cfg.NCH
    r_e = dst // cfg.NPC
    dloc = newlocal[dst]
    b_e = dloc // P
    sg = newglobal[src]
    ch_e = sg // cfg.W
    swin = sg % cfg.W

    key = (r_e * B + b_e) * NCH + ch_e
    order = np.argsort(key, kind="stable")
    cnt = np.bincount(key, minlength=cfg.NCORES * B * NCH)
    cnt = cnt.reshape(cfg.NCORES, B, NCH)
    T_bc = np.maximum(1, -(-cnt.max(axis=0) // P))       # [B, NCH] tiles
    NT_b = T_bc.sum(axis=1)                              # [B]
    starts = np.concatenate([[0], np.cumsum(cnt.reshape(-1))])

    sw_sorted = swin[order]
    dl_sorted = dloc[order]

    pr = Prep()
    pr.newlocal = newlocal
    pr.T_bc, pr.NT_b = T_bc, NT_b

    def build(groups):
        """Per-core gather metadata for a group structure."""
        meta = {
            "groups": groups,
            "NTG": [],            # per group: total tiles
            "NTG_c": [],          # per group: [NCH] tiles per window
            "blk_ranges": [],     # per group: {b: [(tile_off, ntiles)]*NCH}
        }
        for g in groups:
            ntg_c = [int(sum(T_bc[b, c] for b in g)) for c in range(NCH)]
            meta["NTG_c"].append(ntg_c)
            meta["NTG"].append(int(sum(ntg_c)))
            rng = {b: [] for b in g}
            off = 0
            for c in range(NCH):
                for b in g:
                    rng[b].append((off, int(T_bc[b, c])))
                    off += int(T_bc[b, c])
            meta["blk_ranges"].append(rng)

        percore = []
        for r in range(cfg.NCORES):
            sidx_cols, eidx_cols, slot_cols = [], [], []
            for gi, g in enumerate(groups):
                ntg = meta["NTG"][gi]
                e_arr = np.full(ntg * P, cfg.NOWN, np.int64)
                s_arr = np.full(ntg * P, -1, np.int64)
                off = 0
                for c in range(NCH):
                    w_arr = np.zeros(meta["NTG_c"][gi][c] * P, np.int64)
                    coff = 0
                    for b in g:
                        k = (r * B + b) * NCH + c
                        n = starts[k + 1] - starts[k]
                        seg = slice(starts[k], starts[k + 1])
                        w_arr[coff:coff + n] = sw_sorted[seg]
                        e_arr[off + coff:off + coff + n] = dl_sorted[seg]
                        s_arr[off + coff:off + coff + n] = dl_sorted[seg] % P
                        coff += int(T_bc[b, c]) * P
                    off += coff
                    sidx_cols.append(_wrap16(w_arr))
                eidx_cols.append(_wrap16(e_arr))
                slot_cols.append(s_arr.reshape(ntg, P).T)   # [128, ntg]
            slots = np.concatenate(slot_cols, axis=1)        # [128, sumNTG]
            onehot = (slots[:, :, None] ==
                      np.arange(P)[None, None, :]).astype(BF)
            percore.append({
                "sidx": np.concatenate(sidx_cols, axis=1),
                "eidx": np.concatenate(eidx_cols, axis=1),
                "ocat": np.ascontiguousarray(
                    onehot.reshape(P, -1)),
            })
        return meta, percore

    pr.meta1, pr.percore1 = build(_groups(B, NB1))
    pr.meta2, pr.percore2 = build(_groups(B, NB2))
    return pr


def _fold_el(W, a, heads, dph):
    return np.einsum("khd,hd->kh",
                     W.reshape(W.shape[0], heads, dph), a).astype(np.float32)


# ---------------------------------------------------------------- program

def build_layer_program(cfg, pr, layer):
    P, F, F2 = cfg.P, cfg.F, cfg.F2
    H, NCH, B = cfg.HEADS, cfg.NCH, cfg.BLOCKS
    fo = F2 if layer == 2 else F          # 160 / 64
    fin = cfg.IN if layer == 0 else F     # 128 / 64
    ROW = 256 if layer == 2 else 128      # bf16 elems per table row
    meta = pr.meta2 if layer == 2 else pr.meta1
    T_bc = pr.T_bc
    groups = meta["groups"]
    NTG_max = max(meta["NTG"])
    fo4 = fo + 4

    nc = bacc.Bacc("TRN2", target_bir_lowering=False, debug=False,
                   num_devices=cfg.NCORES)

    def inp(name, shape, dt=F32):
        return nc.dram_tensor(name, shape, dt, kind="ExternalInput").ap()

    x_in = inp("x_in", [fin, cfg.NOWN], BF16)
    sidx_in = inp("sidx", [P, sum(meta["NTG"]) * 8], I16)
    eidx_in = inp("eidx", [P, sum(meta["NTG"]) * 8], I16)
    ocat_in = inp("ocat", [P, sum(meta["NTG"]) * P], BF16)
    Wext_i = inp("Wext", [fin, fo + 8], BF16)
    if layer < 2:
        gamma_i = inp("gamma", [F, 1])
        beta_i = inp("beta", [F, 1])
        bsel_in = inp("bsel", [H, F], BF16)
        y_out = nc.dram_tensor("y_out", [F, cfg.NOWN], BF16,
                               kind="ExternalOutput").ap()
    if layer == 0:
        resW_i = inp("resW", [cfg.IN, F], BF16)
    if layer == 2:
        resW_i = inp("resW", [F, F2], BF16)
        b2bc_in = inp("b2bc", [P, F2])
        out = nc.dram_tensor("out", [cfg.NOWN, F2], F32,
                             kind="ExternalOutput").ap()

    rg = [list(range(cfg.NCORES))]

    with tile.TileContext(nc) as tc:
        with (
            tc.tile_pool(name="big", bufs=1) as bigp,
            tc.tile_pool(name="const", bufs=1) as cons,
            tc.tile_pool(name="gat", bufs=2) as gp,
            tc.tile_pool(name="wk", bufs=2) as wk,
            tc.tile_pool(name="ps", bufs=2, space="PSUM") as ps,
            tc.tile_pool(name="ps2", bufs=2, space="PSUM") as ps2,
            tc.tile_pool(name="dram", bufs=1, space="DRAM") as dr,
        ):
            nc.gpsimd.load_library(mlp)

            tbl_own = dr.tile([cfg.NOWN, ROW], BF16)
            tbl_full = dr.tile([cfg.NPAD, ROW], BF16, addr_space="Shared")
            er_dram = dr.tile([cfg.NOWN + 1, 64], F32)
            if layer < 2:
                stats_in = dr.tile([F, 2], F32)
                stats_out = dr.tile([F, 2], F32, addr_space="Shared")

            xT = bigp.tile([fin, cfg.NOWN], BF16, tag="xT")
            nc.sync.dma_start(out=xT[:], in_=x_in[:])
            if layer < 2:
                # rows 0:F raw sums -> h -> y; rows F:F+4 denominators
                hraw = bigp.tile([F + 4, cfg.NOWN], BF16, tag="hraw")

            Wext = cons.tile([fin, fo + 8], BF16, tag="Wext")
            nc.sync.dma_start(out=Wext[:], in_=Wext_i[:])
            if layer < 2:
                # bsel parked at partitions F:F+4 so the recb matmul's lhsT
                # shares hraw's denominator-row base partition
                bsel = cons.tile([F + 4, F], BF16, tag="bsel")
                nc.sync.dma_start(out=bsel[F:F + 4, :], in_=bsel_in[:])
                gam = cons.tile([F, 1], F32, tag="gam")
                bet = cons.tile([F, 1], F32, tag="bet")
                nc.sync.dma_start(out=gam[:], in_=gamma_i[:])
                nc.sync.dma_start(out=bet[:], in_=beta_i[:])
            if layer == 0:
                rW = cons.tile([cfg.IN, F], BF16, tag="rW")
                nc.sync.dma_start(out=rW[:], in_=resW_i[:])
            if layer == 2:
                rW = cons.tile([F, F2], BF16, tag="rW")
                nc.sync.dma_start(out=rW[:], in_=resW_i[:])
                b2bc = cons.tile([P, F2], F32, tag="b2bc")
                nc.sync.dma_start(out=b2bc[:], in_=b2bc_in[:])
            padrow = cons.tile([1, 4], F32, tag="padrow")
            nc.vector.memset(padrow[:], -1e38)

            # ---- phase A: per-node table rows + AllGather ----
            for b in range(B):
                tp = ps.tile([P, fo + 8], F32, tag="tp")
                nc.tensor.matmul(out=tp[:], lhsT=xT[:, b * P:(b + 1) * P],
                                 rhs=Wext[:], start=True, stop=True)
                rowb = wk.tile([P, ROW], BF16, tag="rowb")
                nc.scalar.copy(out=rowb[:, :fo], in_=tp[:, :fo])
                nc.vector.tensor_copy(
                    out=rowb[:, fo:fo + 8].bitcast(F32), in_=tp[:, fo:fo + 4])
                erow = wk.tile([P, 4], F32, tag="erow")
                nc.vector.tensor_copy(out=erow[:], in_=tp[:, fo + 4:fo + 8])
                nc.sync.dma_start(out=tbl_own[b * P:(b + 1) * P, :],
                                  in_=rowb[:])
                nc.sync.dma_start(out=er_dram[b * P:(b + 1) * P, 0:4],
                                  in_=erow[:])
            nc.sync.dma_start(out=er_dram[cfg.NOWN:cfg.NOWN + 1, 0:4],
                              in_=padrow[:])
            nc.gpsimd.collective_compute(
                "AllGather", mybir.AluOpType.bypass, replica_groups=rg,
                ins=[tbl_own[:].opt()], outs=[tbl_full[:].opt()])
            tc.strict_bb_all_engine_barrier()

            # ---- phase B: edge aggregation per group ----
            scol = 0   # running col offset (units of NTG) into idx/ocat
            for gi, g in enumerate(groups):
                ntg = meta["NTG"][gi]
                sidx = gp.tile([P, NTG_max * 8], I16, tag="sidx")
                nc.sync.dma_start(out=sidx[:, :ntg * 8],
                                  in_=sidx_in[:, scol * 8:(scol + ntg) * 8])
                eidx = gp.tile([P, NTG_max * 8], I16, tag="eidx")
                nc.sync.dma_start(out=eidx[:, :ntg * 8],
                                  in_=eidx_in[:, scol * 8:(scol + ntg) * 8])
                O = gp.tile([P, NTG_max * P], BF16, tag="O")
                nc.sync.dma_start(out=O[:, :ntg * P],
                                  in_=ocat_in[:, scol * P:(scol + ntg) * P])

                G = gp.tile([P, NTG_max * ROW], BF16, tag="G")
                G3 = G[:].rearrange("p (t f) -> p t f", f=ROW)
                R = gp.tile([P, NTG_max * 64], F32, tag="R")
                R3 = R[:].rearrange("p (t f) -> p t f", f=64)
                if _STAGE < 1 or gi >= _MAXG:
                    scol += ntg
                    continue
                toff = 0
                coloff = 0
                if not _NOSRC:
                    for c in range(NCH):
                        ntc = meta["NTG_c"][gi][c]
                        if ntc == 0:
                            continue
                        nc.gpsimd.dma_gather(
                            out_ap=G3[:, toff:toff + ntc, :],
                            in_ap=tbl_full[c * cfg.W:(c + 1) * cfg.W, :],
                            idxs_ap=sidx[:, coloff:coloff + ntc * 8],
                            num_idxs=ntc * P, num_idxs_reg=ntc * P,
                            elem_size=ROW, single_packet=False)
                        toff += ntc
                        coloff += ntc * 8
                if not _NOER:
                    nc.gpsimd.dma_gather(
                        out_ap=R3[:, :ntg, :],
                        in_ap=er_dram[:, :],
                        idxs_ap=eidx[:, :ntg * 8],
                        num_idxs=ntg * P, num_idxs_reg=ntg * P,
                        elem_size=64, single_packet=False)
                if _STAGE < 2:
                    scol += ntg
                    continue

                # ee = exp(leaky_relu(el + er)) -> bf16
                eeF = wk.tile([P, NTG_max * H], F32, tag="eeF")
                nc.vector.tensor_tensor(
                    out=eeF[:, :ntg * H].rearrange("p (t h) -> p t h", h=H),
                    in0=G3[:, :ntg, fo:fo + 8].bitcast(F32),
                    in1=R3[:, :ntg, 0:4],
                    op=mybir.AluOpType.add)
                nc.vector.scalar_tensor_tensor(
                    out=eeF[:, :ntg * H], in0=eeF[:, :ntg * H],
                    scalar=cfg.NEG, in1=eeF[:, :ntg * H],
                    op0=mybir.AluOpType.mult, op1=mybir.AluOpType.max)
                eeB = wk.tile([P, NTG_max * H], BF16, tag="eeB")
                nc.scalar.activation(out=eeB[:, :ntg * H],
                                     in_=eeF[:, :ntg * H],
                                     func=mybir.ActivationFunctionType.Exp)
                ee3 = eeB[:, :ntg * H].rearrange("p (t h) -> p t h", h=H)

                # V = [ee * feat | ee]
                V = wk.tile([P, NTG_max * fo4], BF16, tag="V")
                V3 = V[:].rearrange("p (t f) -> p t f", f=fo4)
                nc.vector.tensor_tensor(
                    out=V3[:, :ntg, :fo].rearrange(
                        "p t (h d) -> p t h d", h=H),
                    in0=G3[:, :ntg, :fo].rearrange(
                        "p t (h d) -> p t h d", h=H),
                    in1=ee3.unsqueeze(3).to_broadcast(
                        [P, ntg, H, fo // H]),
                    op=mybir.AluOpType.mult)
                nc.vector.tensor_copy(out=V3[:, :ntg, fo:fo4], in_=ee3)
                if _STAGE < 3:
                    scol += ntg
                    continue

                O3 = O[:].rearrange("p (t s) -> p t s", s=P)
                for b in g:
                    tiles = []
                    for (t0, nt) in meta["blk_ranges"][gi][b]:
                        tiles.extend(range(t0, t0 + nt))
                    if layer < 2:
                        acc = ps.tile([fo4, P], F32, tag="acc")
                        for i, t in enumerate(tiles):
                            nc.tensor.matmul(
                                out=acc[:], lhsT=V3[:, t, :],
                                rhs=O3[:, t, :],
                                start=(i == 0), stop=(i == len(tiles) - 1))
                        nc.scalar.copy(out=hraw[:, b * P:(b + 1) * P],
                                       in_=acc[:])
                    else:
                        acc = ps.tile([P, fo4], F32, tag="acc")
                        for i, t in enumerate(tiles):
                            nc.tensor.matmul(
                                out=acc[:], lhsT=O3[:, t, :],
                                rhs=V3[:, t, :],
                                start=(i == 0), stop=(i == len(tiles) - 1))
                        # normalize + residual + bias, node-major
                        dmax = wk.tile([P, H], F32, tag="dmax")
                        nc.vector.tensor_scalar_max(
                            out=dmax[:], in0=acc[:, fo:fo4], scalar1=1e-16)
                        rcp = wk.tile([P, H], F32, tag="rcp")
                        nc.vector.reciprocal(out=rcp[:], in_=dmax[:])
                        res_ps = ps2.tile([P, F2], F32, tag="res")
                        nc.tensor.matmul(out=res_ps[:],
                                         lhsT=xT[:, b * P:(b + 1) * P],
                                         rhs=rW[:], start=True, stop=True)
                        rst = wk.tile([P, F2], F32, tag="rst")
                        nc.vector.tensor_tensor(
                            out=rst[:].rearrange("p (h o) -> p h o", h=H),
                            in0=acc[:, :fo].rearrange("p (h o) -> p h o", h=H),
                            in1=rcp[:].unsqueeze(2).to_broadcast(
                                [P, H, cfg.OUTC]),
                            op=mybir.AluOpType.mult)
                        nc.vector.tensor_tensor(out=rst[:], in0=rst[:],
                                                in1=res_ps[:],
                                                op=mybir.AluOpType.add)
                        nc.vector.tensor_tensor(out=rst[:], in0=rst[:],
                                                in1=b2bc[:],
                                                op=mybir.AluOpType.add)
                        nc.sync.dma_start(out=out[b * P:(b + 1) * P, :],
                                          in_=rst[:])
                scol += ntg

            # ---- phase C (layers 0/1): normalize + residual + BN ----
            if layer < 2 and _STAGE < 4:
                if _STAGE < 3:
                    nc.vector.memset(hraw[:], 0.0)
                nc.sync.dma_start(out=y_out[:], in_=hraw[0:F, :])
            if layer < 2 and _STAGE >= 4:
                den = hraw[F:F + 4, :]
                nc.vector.tensor_scalar_max(out=den, in0=den, scalar1=1e-16)
                with nc.allow_low_precision(reason="softmax denom in bf16"):
                    nc.vector.reciprocal(out=den, in_=den)
                for b in range(B):
                    recb = ps2.tile([F, P], F32, tag="recb")
                    nc.tensor.matmul(out=recb[:], lhsT=bsel[F:F + 4, :],
                                     rhs=hraw[F:F + 4, b * P:(b + 1) * P],
                                     start=True, stop=True)
                    hsl = hraw[0:F, b * P:(b + 1) * P]
                    nc.vector.tensor_tensor(out=hsl, in0=hsl, in1=recb[:],
                                            op=mybir.AluOpType.mult)
                    if layer == 0:
                        res_ps = ps2.tile([F, P], F32, tag="recb")
                        nc.tensor.matmul(out=res_ps[:], lhsT=rW[:],
                                         rhs=xT[:, b * P:(b + 1) * P],
                                         start=True, stop=True)
                        nc.vector.tensor_tensor(out=hsl, in0=hsl,
                                                in1=res_ps[:],
                                                op=mybir.AluOpType.add)
                    else:
                        nc.vector.scalar_tensor_tensor(
                            out=hsl, in0=xT[:, b * P:(b + 1) * P],
                            scalar=2.0, in1=hsl,
                            op0=mybir.AluOpType.mult,
                            op1=mybir.AluOpType.add)

                if _STAGE < 5:
                    nc.sync.dma_start(out=y_out[:], in_=hraw[0:F, :])
            if layer < 2 and _STAGE >= 5:
                # BN stats over own nodes (pads are zero)
                stats = cons.tile([F, 2], F32, tag="stats")
                half = cfg.NOWN // 2
                sq = bigp.tile([F, half], BF16, tag="sq")
                s2b = cons.tile([F, 1], F32, tag="s2b")
                nc.vector.reduce_sum(out=stats[:, 0:1], in_=hraw[0:F, :],
                                     axis=mybir.AxisListType.X)
                nc.scalar.activation(out=sq[:], in_=hraw[0:F, :half],
                                     func=mybir.ActivationFunctionType.Square)
                nc.vector.reduce_sum(out=stats[:, 1:2], in_=sq[:],
                                     axis=mybir.AxisListType.X)
                nc.scalar.activation(out=sq[:], in_=hraw[0:F, half:],
                                     func=mybir.ActivationFunctionType.Square)
                nc.vector.reduce_sum(out=s2b[:], in_=sq[:],
                                     axis=mybir.AxisListType.X)
                nc.vector.tensor_tensor(out=stats[:, 1:2], in0=stats[:, 1:2],
                                        in1=s2b[:], op=mybir.AluOpType.add)
                if _STAGE == 5:
                    nc.sync.dma_start(out=y_out[:], in_=hraw[0:F, :])
            if layer < 2 and _STAGE >= 6:
                nc.sync.dma_start(out=stats_in[:], in_=stats[:])
                nc.gpsimd.collective_compute(
                    "AllReduce", mybir.AluOpType.add, replica_groups=rg,
                    ins=[stats_in[:].opt()], outs=[stats_out[:].opt()])
                tc.strict_bb_all_engine_barrier()
                ssb = cons.tile([F, 2], F32, tag="ssb")
                nc.sync.dma_start(out=ssb[:], in_=stats_out[:])
                if _STAGE == 6:
                    nc.sync.dma_start(out=y_out[:], in_=hraw[0:F, :])
            if layer < 2 and _STAGE >= 7:
                mu = cons.tile([F, 1], F32, tag="mu")
                tmp1 = cons.tile([F, 1], F32, tag="tmp1")
                scl = cons.tile([F, 1], F32, tag="scl")
                bia = cons.tile([F, 1], F32, tag="bia")
                musq = cons.tile([F, 1], F32, tag="musq")
                invn = 1.0 / cfg.N
                nc.vector.tensor_scalar_mul(out=mu[:], in0=ssb[:, 0:1],
                                            scalar1=invn)
                nc.vector.tensor_scalar_mul(out=tmp1[:], in0=ssb[:, 1:2],
                                            scalar1=invn)
                nc.scalar.activation(out=musq[:], in_=mu[:],
                                     func=mybir.ActivationFunctionType.Square)
                nc.vector.tensor_tensor(out=tmp1[:], in0=tmp1[:], in1=musq[:],
                                        op=mybir.AluOpType.subtract)
                nc.vector.tensor_scalar_add(out=tmp1[:], in0=tmp1[:],
                                            scalar1=cfg.EPS)
                nc.scalar.activation(out=tmp1[:], in_=tmp1[:],
                                     func=mybir.ActivationFunctionType.Sqrt)
                nc.vector.reciprocal(out=tmp1[:], in_=tmp1[:])
                nc.vector.tensor_tensor(out=scl[:], in0=tmp1[:], in1=gam[:],
                                        op=mybir.AluOpType.mult)
                nc.vector.tensor_tensor(out=tmp1[:], in0=mu[:], in1=scl[:],
                                        op=mybir.AluOpType.mult)
                nc.vector.tensor_tensor(out=bia[:], in0=bet[:], in1=tmp1[:],
                                        op=mybir.AluOpType.subtract)
                nc.scalar.activation(out=hraw[0:F, :], in_=hraw[0:F, :],
                                     func=mybir.ActivationFunctionType.Relu,
                                     scale=scl[:, 0:1], bias=bia[:, 0:1])
                if cfg.NPC < cfg.NOWN:
                    nc.vector.memset(hraw[0:F, cfg.NPC:cfg.NOWN], 0.0)
                nc.sync.dma_start(out=y_out[:], in_=hraw[0:F, :])

    nc.compile()
    return nc


# ---------------------------------------------------------------- host glue

def make_in_maps(cfg, inputs, pr):
    H, D, OC = cfg.HEADS, cfg.HID, cfg.OUTC
    feat = np.asarray(inputs["feat"], np.float32)
    W0 = np.asarray(inputs["W0"], np.float32)
    W1 = np.asarray(inputs["W1"], np.float32)
    W2 = np.asarray(inputs["W2"], np.float32)
    W0ext = np.concatenate(
        [W0, _fold_el(W0, np.asarray(inputs["al0"]), H, D),
         _fold_el(W0, np.asarray(inputs["ar0"]), H, D)], axis=1).astype(BF)
    W1ext = np.concatenate(
        [W1, _fold_el(W1, np.asarray(inputs["al1"]), H, D),
         _fold_el(W1, np.asarray(inputs["ar1"]), H, D)], axis=1).astype(BF)
    W2ext = np.concatenate(
        [W2, _fold_el(W2, np.asarray(inputs["al2"]), H, OC),
         _fold_el(W2, np.asarray(inputs["ar2"]), H, OC)], axis=1).astype(BF)
    bsel = np.repeat(np.eye(H, dtype=np.float32), D, axis=1).astype(BF)

    layer_maps = [[], [], []]
    for r in range(cfg.NCORES):
        ids = np.arange(r * cfg.NPC, (r + 1) * cfg.NPC)
        fp = np.zeros((cfg.NOWN, cfg.IN), np.float32)
        fp[pr.newlocal[ids]] = feat[ids]
        idx1 = {k: pr.percore1[r][k] for k in ("sidx", "eidx", "ocat")}
        idx2 = {k: pr.percore2[r][k] for k in ("sidx", "eidx", "ocat")}
        layer_maps[0].append({
            "x_in": np.ascontiguousarray(fp.T).astype(BF), "Wext": W0ext,
            "resW": np.asarray(inputs["resW0"], np.float32).astype(BF),
            "gamma": np.asarray(inputs["gamma0"],
                                np.float32).reshape(cfg.F, 1),
            "beta": np.asarray(inputs["beta0"],
                               np.float32).reshape(cfg.F, 1),
            "bsel": bsel, **idx1,
        })
        layer_maps[1].append({
            "Wext": W1ext,
            "gamma": np.asarray(inputs["gamma1"],
                                np.float32).reshape(cfg.F, 1),
            "beta": np.asarray(inputs["beta1"],
                               np.float32).reshape(cfg.F, 1),
            "bsel": bsel, **idx1,
        })
        layer_maps[2].append({
            "Wext": W2ext,
            "resW": np.asarray(inputs["resW2"], np.float32).astype(BF),
            "b2bc": np.tile(np.asarray(inputs["b2"],
                                       np.float32).reshape(1, cfg.F2),
                            (cfg.P, 1)),
            **idx2,
        })
    return layer_maps


def assemble_output(cfg, results, pr):
    out = np.empty((cfg.N, cfg.F2), np.float32)
    for r in range(cfg.NCORES):
        ids = np.arange(r * cfg.NPC, (r + 1) * cfg.NPC)
        out[ids] = results[r]["out"][pr.newlocal[ids]]
    return out


_PROG_CACHE = {}


def get_program(cfg, pr, layer):
    key = (layer, pr.T_bc.tobytes())
    if key not in _PROG_CACHE:
        _PROG_CACHE[key] = build_layer_program(cfg, pr, layer)
    return _PROG_CACHE[key]


def run(inputs, trace=False, trace_cores=None):
    cfg = CFG
    pr = preprocess(cfg, np.asarray(inputs["src"]), np.asarray(inputs["dst"]))
    layer_maps = make_in_maps(cfg, inputs, pr)
    cores = list(range(cfg.NCORES))
    total_ns = 0
    layer_res = []
    for layer in range(3):
        nc = get_program(cfg, pr, layer)
        res = bass_utils.run_bass_kernel_spmd(
            nc, layer_maps[layer], core_ids=cores,
            trace=trace, trace_cores=trace_cores)
        layer_res.append(res)
        if res.exec_time_ns:
            total_ns += res.exec_time_ns
        if layer < 2:
            for r in range(cfg.NCORES):
                layer_maps[layer + 1][r]["x_in"] = res.results[r]["y_out"]
    out = assemble_output(cfg, layer_res[2].results, pr)
    return out, (total_ns, layer_res)


def kernel(**inputs) -> np.ndarray:
    return run(inputs)[0]
